# revision 1
# baseline (speedup 1.0000x reference)
"""ClusterKLLoss Trainium2 kernel (8 NeuronCores, data-parallel over rows of c_i).

Math (derived from the reference):
  loss = CE(logits, arange(B), sum) / B  with logits[i,j] = -kl[i,j]/T
  kl[i,j] = hneg[j] - Li[i] . Q[j],  Q = softmax(c_j), hneg[j] = sum Q log Q.
  Per-row (i) constant shifts cancel in log-softmax, so log_softmax(c_i) is
  never needed:
    G[i,j] = (c_i[i] . Q[j] - hneg[j]) / T       (logits up to per-row shift)
  With E = exp(c_j) (no max-sub needed for N(0,1) inputs), Z_j = sum_k E[j,k],
  A_j = sum_k E[j,k] c_j[j,k]:
    hneg_j = A_j/Z_j - ln Z_j
    G[i,j] = (S[i,j] + e_j) * s_j,  S = c_i @ E^T,  e_j = Z_j ln Z_j - A_j,
    s_j = 1/(T Z_j)
  loss = sum_i (logsumexp_j G[i,j] - G[i,i]) / B

Sharding: core c takes c_i rows [512c, 512c+512) and a rotated copy of c_j
(np.roll(c_j, -512c, axis=0)) so the diagonal lands at local columns
[0, 512) on every core -> one SPMD NEFF, no per-core addressing. Row
logsumexp is permutation-invariant so the rotation changes nothing else.
Each core returns its scalar partial; the host sums 8 partials / B.
"""

import sys

for _p in ("/opt/trn_rl_repo",):
    if _p not in sys.path:
        sys.path.insert(0, _p)

import numpy as np

import concourse.bass as bass
import concourse.bacc as bacc
import concourse.tile as tile
from concourse import mybir
from concourse import bass_utils

B = 4096
D = 2048
TEMP = 0.5
NCORES = 8
SHARD = B // NCORES  # 512
KT = D // 128  # 16 k partition-tiles
NCH = 8  # 512-wide column chunks
F32 = mybir.dt.float32
F16 = mybir.dt.float16
AF = mybir.ActivationFunctionType
OP = mybir.AluOpType
AX = mybir.AxisListType

NEG_INF = -3.0e38

import os
LOADS_GPSIMD = os.environ.get("K_LOADS_GPSIMD", "0") == "1"
XPOSE_SCALAR = os.environ.get("K_XPOSE_SCALAR", "0") == "1"
PROD_MOD = int(os.environ.get("K_PROD_MOD", "4"))


CSCALE = 4096.0  # power-of-two normalizer keeping W' = E*s*C in fp16 normal range
INV_C = 1.0 / CSCALE


def build_kernel_body(tc, out_ap, ci_ap, cj_ap, eye_ap, reps=1):
    """Emit the kernel IR. out: [1,1] f32; ci: [512,2048] f32;
    cj: [4096,2048] f32 (rotated per-core); eye: [128,128] f32.

    v2: per-j softmax scale is folded into the fp16 rhs operand
    (W' = E * C/(T*Z_j)), the bias row rides the matmul as two fp16 hi/lo
    K-rows, and the row-softmax needs no max subtraction (G in [-11, 27]),
    so ACT consumes PSUM directly: exp(S*2^-12) with free row-sum accum.
    """
    nc = tc.nc

    from contextlib import ExitStack

    with ExitStack() as ctx:
        singles = ctx.enter_context(tc.tile_pool(name="singles", bufs=1))
        xpool = ctx.enter_context(tc.tile_pool(name="xpool", bufs=3))
        epool = ctx.enter_context(tc.tile_pool(name="epool", bufs=3))
        etpool = ctx.enter_context(tc.tile_pool(name="etpool", bufs=5))
        spool = ctx.enter_context(tc.tile_pool(name="spool", bufs=3))
        psS = ctx.enter_context(tc.tile_pool(name="psS", bufs=6, space="PSUM"))
        psX = ctx.enter_context(tc.tile_pool(name="psX", bufs=2, space="PSUM"))

        # constants
        eye32 = singles.tile([128, 128], F32)
        nc.sync.dma_start(out=eye32, in_=eye_ap)
        eye16 = singles.tile([128, 128], F16)
        nc.vector.tensor_copy(out=eye16, in_=eye32)
        ones2 = singles.tile([2, 128], F16)
        nc.vector.memset(ones2, 1.0)
        onesc = singles.tile([128, 1], F32)
        nc.vector.memset(onesc, 1.0)

        # per-j scalar accumulators (col t = j-tile t)
        Zc = singles.tile([128, 32], F32)
        Ac = singles.tile([128, 32], F32)
        sCc = singles.tile([128, 32], F32)
        Zparts = singles.tile([128, 32], F32)  # col = m*8 + n
        Dc = singles.tile([128, 4], F32)
        Zi = singles.tile([128, 4], F32)

        # ci -> fp16 -> transposed [k-part, i] layout
        ciT = singles.tile([128, 4, KT, 128], F16)
        for t in range(4):
            cit = xpool.tile([128, D], F32, tag="xload")
            nc.sync.dma_start(out=cit, in_=ci_ap[128 * t : 128 * (t + 1), :])
            c16 = epool.tile([128, D], F16, tag="estg")
            nc.vector.tensor_copy(out=c16, in_=cit)
            nc.sync.dma_start_transpose(out=ciT[:, t], in_=c16)

        for _rep in range(reps):
            _run_main(tc, ctx, out_ap, cj_ap, locals())


def _run_main(tc, ctx, out_ap, cj_ap, env):
    nc = tc.nc
    singles = env["singles"]; xpool = env["xpool"]; epool = env["epool"]
    etpool = env["etpool"]; spool = env["spool"]; psS = env["psS"]; psX = env["psX"]
    eye32 = env["eye32"]; eye16 = env["eye16"]; ones2 = env["ones2"]; onesc = env["onesc"]
    Zc = env["Zc"]; Ac = env["Ac"]; sCc = env["sCc"]; Zparts = env["Zparts"]
    Dc = env["Dc"]; Zi = env["Zi"]; ciT = env["ciT"]
    if True:
        for n in range(NCH):
            ETc = etpool.tile([128, 4, KT, 128], F16, tag="et")
            for q in range(4):
                t = 4 * n + q
                xt = xpool.tile([128, D], F32, tag="xload")
                (nc.gpsimd if LOADS_GPSIMD else nc.sync).dma_start(
                    out=xt, in_=cj_ap[128 * t : 128 * (t + 1), :]
                )
                es = epool.tile([128, D], F16, tag="estg")
                # E = exp(x); Z_j accumulated for free
                nc.scalar.activation(
                    out=es, in_=xt, func=AF.Exp, accum_out=Zc[:, t : t + 1]
                )
                # A_j = sum_k E*x: product (split DVE/GPSIMD) + DVE reduce
                prod = epool.tile([128, D], F16, tag="prod")
                if t % PROD_MOD == 0:
                    nc.vector.tensor_mul(prod, es, xt)
                else:
                    nc.gpsimd.tensor_mul(prod, es, xt)
                nc.vector.tensor_reduce(
                    out=Ac[:, t : t + 1], in_=prod, axis=AX.X, op=OP.add
                )
                # sC_j = C/(T*Z_j); W' = E*sC in fp16 (normal range)
                nc.vector.tensor_scalar_mul(
                    sCc[:, t : t + 1], Zc[:, t : t + 1], float(TEMP / CSCALE)
                )
                nc.vector.reciprocal(
                    out=sCc[:, t : t + 1], in_=sCc[:, t : t + 1]
                )
                ws = epool.tile([128, D], F16, tag="ws")
                nc.vector.tensor_scalar_mul(ws, es, sCc[:, t : t + 1])
                # W'^T into this chunk's rhs tile (contiguous 3D dest)
                (nc.scalar if XPOSE_SCALAR else nc.sync).dma_start_transpose(
                    out=ETc[:, q], in_=ws
                )

            # per-chunk bias row: b' = (lnZ - A/Z)*(C/T) = lnZ*(C/T) - A*sC
            z4 = Zc[:, 4 * n : 4 * n + 4]
            a4 = Ac[:, 4 * n : 4 * n + 4]
            lnz = spool.tile([128, 4], F32, tag="lnz")
            nc.scalar.activation(out=lnz, in_=z4, func=AF.Ln)
            bp = spool.tile([128, 4], F32, tag="bp")
            nc.vector.tensor_mul(bp, a4, sCc[:, 4 * n : 4 * n + 4])
            lnzs = spool.tile([128, 4], F32, tag="lnzs")
            nc.vector.tensor_scalar_mul(lnzs, lnz, float(CSCALE / TEMP))
            nc.vector.tensor_sub(bp, lnzs, bp)
            # split bias into fp16 hi+lo (keeps fp32 accuracy in the matmul)
            e2 = spool.tile([128, 4, 2], F16, tag="e2")
            nc.vector.tensor_copy(out=e2[:, :, 0], in_=bp)
            nc.vector.tensor_sub(e2[:, :, 1], bp, e2[:, :, 0])
            # transpose per q and collect into one [2, 512] rhs row pair
            e2row = spool.tile([2, 512], F16, tag="e2row")
            for q in range(4):
                e2q_ps = psX.tile([2, 128], F16, tag="xp", bufs=1, name=f"e2ps{n}_{q}")
                nc.tensor.transpose(e2q_ps, e2[:, q, :], eye16)
                nc.vector.tensor_copy(
                    out=e2row[:, 128 * q : 128 * (q + 1)], in_=e2q_ps
                )

            # main matmuls; ACT consumes PSUM directly (exp + row-sum accum)
            for m in range(4):
                S_ps = psS.tile([128, 512], F32, tag="s")
                for kt in range(KT):
                    nc.tensor.matmul(
                        S_ps,
                        ciT[:, m, kt, :],
                        ETc[:, :, kt, :],
                        start=(kt == 0),
                        stop=False,
                    )
                nc.tensor.matmul(S_ps, ones2, e2row, start=False, stop=True)
                if n == 0:
                    junk = spool.tile([128, 128], F32, tag="junk")
                    nc.vector.tensor_mul(
                        junk, S_ps[:, 128 * m : 128 * (m + 1)], eye32
                    )
                    nc.vector.tensor_reduce(
                        out=Dc[:, m : m + 1], in_=junk, axis=AX.X, op=OP.add
                    )
                expj = spool.tile([128, 512], F16, tag="expj", bufs=2)
                nc.scalar.activation(
                    out=expj,
                    in_=S_ps,
                    func=AF.Exp,
                    scale=float(INV_C),
                    accum_out=Zparts[:, 8 * m + n : 8 * m + n + 1],
                )

        # lse_i = ln(sum_n Zparts); loss terms = lse - diag*2^-12
        Zp = Zparts.rearrange("p (m n) -> p m n", n=8)
        nc.vector.tensor_reduce(out=Zi, in_=Zp, axis=AX.X, op=OP.add)
        lnzi = spool.tile([128, 4], F32, tag="lnzi")
        nc.scalar.activation(out=lnzi, in_=Zi, func=AF.Ln)
        gd = spool.tile([128, 4], F32, tag="gd")
        nc.vector.tensor_scalar_mul(gd, Dc, float(INV_C))
        terms = spool.tile([128, 4], F32, tag="terms")
        nc.vector.tensor_sub(terms, lnzi, gd)
        part_ps = psX.tile([1, 4], F32, tag="xp", bufs=1)
        nc.tensor.matmul(part_ps, onesc, terms, start=True, stop=True)
        part = spool.tile([1, 4], F32, tag="part")
        nc.vector.tensor_copy(out=part, in_=part_ps)
        res = spool.tile([1, 1], F32, tag="res")
        nc.vector.reduce_sum(out=res, in_=part, axis=AX.X)
        nc.sync.dma_start(out=out_ap, in_=res)


_NC_CACHE = {}


def build_nc(reps=1):
    key = ("nc", reps)
    if key in _NC_CACHE:
        return _NC_CACHE[key]
    nc = bacc.Bacc("TRN2", target_bir_lowering=False, debug=False)
    ci = nc.dram_tensor("ci", [SHARD, D], F32, kind="ExternalInput").ap()
    cj = nc.dram_tensor("cj", [B, D], F32, kind="ExternalInput").ap()
    eye = nc.dram_tensor("eye", [128, 128], F32, kind="ExternalInput").ap()
    out = nc.dram_tensor("out", [1, 1], F32, kind="ExternalOutput").ap()
    with tile.TileContext(nc) as tc:
        build_kernel_body(tc, out, ci, cj, eye, reps=reps)
    nc.compile()
    _NC_CACHE[key] = nc
    return nc


def make_in_maps(c_i, c_j):
    eye = np.eye(128, dtype=np.float32)
    in_maps = []
    for c in range(NCORES):
        in_maps.append(
            {
                "ci": np.ascontiguousarray(c_i[SHARD * c : SHARD * (c + 1)]),
                "cj": np.ascontiguousarray(np.roll(c_j, -SHARD * c, axis=0)),
                "eye": eye,
            }
        )
    return in_maps


def kernel(c_i, c_j, **kwargs):
    c_i = np.ascontiguousarray(np.asarray(c_i, dtype=np.float32))
    c_j = np.ascontiguousarray(np.asarray(c_j, dtype=np.float32))
    nc = build_nc()
    in_maps = make_in_maps(c_i, c_j)
    res = bass_utils.run_bass_kernel_spmd(
        nc, in_maps, core_ids=list(range(NCORES))
    )
    total = np.float64(0.0)
    for r in res.results:
        total += np.float64(r["out"][0, 0])
    return np.float32(total / B).reshape(())



# revision 18
# speedup vs baseline: 2.5058x; 2.5058x over previous
"""ClusterKLLoss Trainium2 kernel (8 NeuronCores, j-sharded, fp8 DoubleRow).

Math (from the reference):
  loss = CE(logits, arange(B), sum)/B, logits[i,j] = -kl[i,j]/T
  kl[i,j] = hneg[j] - Li[i].Q[j], Q = softmax(c_j), hneg[j] = sum Q logQ.
  Per-row-i shifts cancel in log-softmax, so with E = exp(c_j),
  Z_j = sum E, A_j = sum E*c_j, and T = 1/2:
    G[i,j] = (c_i[i].Q_j)/T + 2 lnZ_j - 2 A_j/Z_j   (logits + per-i const)
    loss   = sum_i [logsumexp_j G[i,j] - G[i,i]] / B

Sharding: core c owns c_j rows [512c, 512c+512) (4 partition-tiles of 128 j)
and the FULL c_i as a host-transposed fp8 ciT [2048, 4096].  Each core
computes stripes S'[j=128, i=512] = sum_k W8[j,k] ciT[k,i] with
W8 = E * (CW/Z_j) in fp8 (DoubleRow, K=256/matmul).  The scalar engine then
forms u = exp(S'/4096 + bb_j) with per-partition bias bb_j = -2A/Z - b0c,
the DVE scales by zn2_j = (Z_j/Z0)^2 and accumulates over the 4 j-tiles into
zp[j=128, i=4096] (fp16).  Host: Zi = sum over cores+partitions of zp,
lse_i = ln(Zi) + b0c + 2 ln Z0; diagonal terms are reconstructed exactly on
the host from per-j Z, A and dsum_j = sum_k c_i[j,k] E[j,k] (all fp32).
b0c/Z0 are fixed rescalings keeping fp16 in range; they cancel exactly.
"""

import sys

for _p in ("/opt/trn_rl_repo",):
    if _p not in sys.path:
        sys.path.insert(0, _p)

import numpy as np
import ml_dtypes

import concourse.bass as bass
import concourse.bacc as bacc
import concourse.tile as tile
from concourse import mybir
from concourse import bass_utils

B = 4096
D = 2048
TEMP = 0.5
NCORES = 8
SHARD = B // NCORES  # 512 j-rows per core
NT = SHARD // 128  # 4 j partition-tiles
KT = D // 128  # 16 k partition-tiles
NCH = 8  # i chunks of 512

F32 = mybir.dt.float32
F16 = mybir.dt.float16
F8 = mybir.dt.float8e4
AF = mybir.ActivationFunctionType
OP = mybir.AluOpType
AX = mybir.AxisListType
PM = mybir.MatmulPerfMode

import os

USE_DR = os.environ.get("K_DR", "1") == "1"  # DoubleRow fp8 matmuls

CW = 512.0  # W8 = CW*Q: typ ~0.25, max ~15; fp8e4 max is 240
SINV = 2.0 / CW  # S' * SINV = (ci.Q)/T
B0C = -2.0  # bias recenter: u = exp(G - 2lnZ - B0C) stays ~1 in fp16
Z0 = 3400.0  # zn2 = (Z/Z0)^2 ~ 1


def build_kernel_body(tc, zp_ap, zo_ap, ao_ap, do_ap, cit_ap, cj_ap, ci_ap):
    nc = tc.nc
    from contextlib import ExitStack

    with ExitStack() as ctx:
        singles = ctx.enter_context(tc.tile_pool(name="singles", bufs=1))
        xin = ctx.enter_context(tc.tile_pool(name="xin", bufs=2))
        cin = ctx.enter_context(tc.tile_pool(name="cin", bufs=2))
        est = ctx.enter_context(tc.tile_pool(name="est", bufs=2))
        scr = ctx.enter_context(tc.tile_pool(name="scr", bufs=2))
        wst = ctx.enter_context(tc.tile_pool(name="wst", bufs=2))
        wtt = ctx.enter_context(tc.tile_pool(name="wtt", bufs=2))
        ups = ctx.enter_context(tc.tile_pool(name="ups", bufs=3))
        pps = ctx.enter_context(tc.tile_pool(name="pps", bufs=2))
        psS = ctx.enter_context(tc.tile_pool(name="psS", bufs=4, space="PSUM"))

        # resident tiles
        cit8 = singles.tile([128, KT, B], F8)  # 64KB/part
        wt8 = singles.tile([128, NT, KT, 128], F8)  # 8KB/part
        pacc = singles.tile([128, NCH, 512], F16)  # 8KB/part
        Zc = singles.tile([128, NT], F32)
        Ac = singles.tile([128, NT], F32)
        dsum = singles.tile([128, NT], F32)
        rzc = singles.tile([128, NT], F32)
        scw = singles.tile([128, NT], F32)
        bb2 = singles.tile([128, NT], F32)
        zn2 = singles.tile([128, NT], F32)
        tmp1 = singles.tile([128, NT], F32)

        # ciT: 8 one-shot DMAs on the sync ring.  Host layout [1024, 8192] is
        # eighth-major: row e*128+p holds ciT[kt*128+p, 512e:512e+512] for all
        # kt contiguously -> 8KB/partition contiguous lines, full DMA BW, and
        # i-chunk c2 becomes available as soon as DMA c2 lands.
        for e in range(NCH):
            nc.sync.dma_start(
                out=cit8[:, :, 512 * e : 512 * (e + 1)],
                in_=cit_ap[128 * e : 128 * (e + 1), :],
            )

        # cj/ci loads ride the scalar ring (interleaved with the transposes)
        cj_tiles = [None] * NT
        ci_tiles = [None] * NT

        def load_cj(t):
            cj_tiles[t] = xin.tile([128, D], F32, tag="cj", name=f"cjt{t}")
            nc.scalar.dma_start(
                out=cj_tiles[t], in_=cj_ap[128 * t : 128 * (t + 1), :]
            )

        def load_ci(t):
            ci_tiles[t] = cin.tile([128, D], F32, tag="ci", name=f"cit{t}")
            nc.scalar.dma_start(
                out=ci_tiles[t], in_=ci_ap[128 * t : 128 * (t + 1), :]
            )

        load_cj(0)

        for t in range(NT):
            cj_t = cj_tiles[t]

            # E = exp(cj); Z accumulated for free
            E32 = est.tile([128, D], F32, tag="e32")
            nc.scalar.activation(
                out=E32, in_=cj_t, func=AF.Exp, accum_out=Zc[:, t : t + 1]
            )
            # A_j = sum E*cj
            j1 = scr.tile([128, D], F16, tag="j1")
            nc.vector.tensor_mul(j1, E32, cj_t)
            nc.vector.tensor_reduce(
                out=Ac[:, t : t + 1], in_=j1, axis=AX.X, op=OP.add
            )
            # per-j scalars: rz = 1/Z, scw = CW/Z, bb2 = -2A/Z - B0C,
            # zn2 = (Z/Z0)^2
            nc.vector.reciprocal(out=rzc[:, t : t + 1], in_=Zc[:, t : t + 1])
            nc.vector.tensor_scalar_mul(
                scw[:, t : t + 1], rzc[:, t : t + 1], float(CW)
            )
            nc.vector.tensor_mul(bb2[:, t : t + 1], Ac[:, t : t + 1], rzc[:, t : t + 1])
            nc.vector.tensor_scalar(
                out=bb2[:, t : t + 1], in0=bb2[:, t : t + 1],
                scalar1=-2.0, scalar2=-float(B0C), op0=OP.mult, op1=OP.add,
            )
            nc.vector.tensor_scalar_mul(
                tmp1[:, t : t + 1], Zc[:, t : t + 1], float(1.0 / Z0)
            )
            nc.vector.tensor_mul(
                zn2[:, t : t + 1], tmp1[:, t : t + 1], tmp1[:, t : t + 1]
            )
            # W16 = E * (CW/Z_j) -> transpose -> fp8
            w16 = wst.tile([128, D], F16, tag="w16")
            nc.vector.tensor_scalar_mul(w16, E32, scw[:, t : t + 1])
            wt16 = wtt.tile([128, KT, 128], F16, tag="wt16")
            nc.scalar.dma_start_transpose(out=wt16, in_=w16)
            nc.vector.tensor_copy(out=wt8[:, t], in_=wt16)

            # prefetch next cj, then this tile's ci (diag only, off the
            # critical path); dsum_j = sum ci*E
            if t + 1 < NT:
                load_cj(t + 1)
            load_ci(t)
            ci_t = ci_tiles[t]
            j2 = scr.tile([128, D], F16, tag="j2")
            nc.gpsimd.tensor_mul(j2, E32, ci_t)
            nc.vector.tensor_reduce(
                out=dsum[:, t : t + 1], in_=j2, axis=AX.X, op=OP.add
            )

            # main stripes: S'[j=128, i=512], K=2048 via 8 DoubleRow matmuls
            for c2 in range(NCH):
                S_ps = psS.tile([128, 512], F32, tag="s")
                if USE_DR:
                    for k2 in range(KT // 2):
                        nc.tensor.matmul(
                            S_ps,
                            wt8[:, t, 2 * k2 : 2 * k2 + 2, :],
                            cit8[:, 2 * k2 : 2 * k2 + 2, 512 * c2 : 512 * (c2 + 1)],
                            start=(k2 == 0),
                            stop=(k2 == KT // 2 - 1),
                            perf_mode=PM.DoubleRow,
                        )
                else:
                    for kt in range(KT):
                        nc.tensor.matmul(
                            S_ps,
                            wt8[:, t, kt, :],
                            cit8[:, kt, 512 * c2 : 512 * (c2 + 1)],
                            start=(kt == 0),
                            stop=(kt == KT - 1),
                        )
                u16 = ups.tile([128, 512], F16, tag="u16")
                nc.scalar.activation(
                    out=u16, in_=S_ps, func=AF.Exp,
                    scale=float(SINV), bias=bb2[:, t : t + 1],
                )
                if t == 0:
                    nc.vector.tensor_scalar_mul(
                        pacc[:, c2], u16, zn2[:, t : t + 1]
                    )
                else:
                    p16 = pps.tile([128, 512], F16, tag="p16")
                    nc.vector.tensor_scalar_mul(p16, u16, zn2[:, t : t + 1])
                    nc.vector.tensor_add(pacc[:, c2], pacc[:, c2], p16)
                if t == NT - 1:
                    nc.sync.dma_start(
                        out=zp_ap[:, 512 * c2 : 512 * (c2 + 1)], in_=pacc[:, c2]
                    )

        nc.sync.dma_start(out=zo_ap, in_=Zc)
        nc.sync.dma_start(out=ao_ap, in_=Ac)
        nc.sync.dma_start(out=do_ap, in_=dsum)


_NC_CACHE = {}


def build_nc():
    key = "nc"
    if key in _NC_CACHE:
        return _NC_CACHE[key]
    nc = bacc.Bacc("TRN2", target_bir_lowering=False, debug=False)
    cit = nc.dram_tensor("cit", [NCH * 128, KT * 512], F8, kind="ExternalInput").ap()
    cj = nc.dram_tensor("cj", [SHARD, D], F32, kind="ExternalInput").ap()
    ci = nc.dram_tensor("ci", [SHARD, D], F32, kind="ExternalInput").ap()
    zp = nc.dram_tensor("zp", [128, B], F16, kind="ExternalOutput").ap()
    zo = nc.dram_tensor("zo", [128, NT], F32, kind="ExternalOutput").ap()
    ao = nc.dram_tensor("ao", [128, NT], F32, kind="ExternalOutput").ap()
    do = nc.dram_tensor("do", [128, NT], F32, kind="ExternalOutput").ap()
    with tile.TileContext(nc) as tc:
        build_kernel_body(tc, zp, zo, ao, do, cit, cj, ci)
    nc.compile()
    _NC_CACHE[key] = nc
    return nc


def make_in_maps(c_i, c_j):
    # ciT in eighth-major layout: row e*128+p, col kt*512+i' holds
    # ciT[kt*128+p, 512e+i'] = c_i[512e+i', kt*128+p]
    cit8 = c_i.T.astype(ml_dtypes.float8_e4m3)  # [D, B] = [(kt p), (e i')]
    cit8 = cit8.reshape(KT, 128, NCH, 512).transpose(2, 1, 0, 3)
    cit8 = np.ascontiguousarray(cit8).reshape(NCH * 128, KT * 512)
    in_maps = []
    for c in range(NCORES):
        in_maps.append(
            {
                "cit": cit8,
                "cj": np.ascontiguousarray(c_j[SHARD * c : SHARD * (c + 1)]),
                "ci": np.ascontiguousarray(c_i[SHARD * c : SHARD * (c + 1)]),
            }
        )
    return in_maps


def kernel(c_i, c_j, **kwargs):
    c_i = np.ascontiguousarray(np.asarray(c_i, dtype=np.float32))
    c_j = np.ascontiguousarray(np.asarray(c_j, dtype=np.float32))
    nc = build_nc()
    in_maps = make_in_maps(c_i, c_j)
    res = bass_utils.run_bass_kernel_spmd(
        nc, in_maps, core_ids=list(range(NCORES))
    )

    Zi = np.zeros(B, dtype=np.float64)
    gii_sum = np.float64(0.0)
    for r in res.results:
        Zi += r["zp"].astype(np.float64).sum(axis=0)
        Z = r["zo"].astype(np.float64)
        A = r["ao"].astype(np.float64)
        ds = r["do"].astype(np.float64)
        # G_ii for this core's 512 j-rows, exact in f64
        gii = ds / (TEMP * Z) + 2.0 * np.log(Z) - 2.0 * A / Z
        gii_sum += gii.sum()
    lse = np.log(Zi) + B0C + 2.0 * np.log(Z0)
    loss = (lse.sum() - gii_sum) / B
    return np.float32(loss).reshape(())


# revision 20
# speedup vs baseline: 2.7869x; 1.1122x over previous
"""ClusterKLLoss Trainium2 kernel (8 NeuronCores, j-sharded, fp8 DoubleRow).

Math (from the reference):
  loss = CE(logits, arange(B), sum)/B, logits[i,j] = -kl[i,j]/T
  kl[i,j] = hneg[j] - Li[i].Q[j], Q = softmax(c_j), hneg[j] = sum Q logQ.
  Per-row-i shifts cancel in log-softmax, so with E = exp(c_j),
  Z_j = sum E, A_j = sum E*c_j, and T = 1/2:
    G[i,j] = (c_i[i].Q_j)/T + 2 lnZ_j - 2 A_j/Z_j   (logits + per-i const)
    loss   = sum_i [logsumexp_j G[i,j] - G[i,i]] / B

Sharding: core c owns c_j rows [512c, 512c+512) (4 partition-tiles of 128 j)
and the FULL c_i as a host-transposed fp8 ciT.  Each core computes stripes
S'[j=128, i=512] = sum_k W8[j,k] ciT[k,i] with W8 = E*(CW/Z_j) in fp8
(DoubleRow, K=256/matmul).  The scalar engine forms u = exp(S'/256 + bb_j)
with per-partition bias bb_j = -2A/Z - b0c (b0c keeps u ~1 in fp16), the
DVE scales by zn2_j = (Z_j/Z0)^2 and accumulates the 4 j-tiles into
zp[j=128, i-chunks] fp16.  Host sums zp over cores+partitions into Zi and
lse_i = ln Zi + b0c + 2 ln Z0.

ciT rides in an eighth-major host layout [1024, 8192] (row e*128+p, col
kt*512+i') ROLLED per core so local i-chunk 0 is the core's own j-range:
the diagonal G_ii comes straight out of chunk-0 stripes via an eye mask
(gd = diag(S')*SINV + bb_j = G_ii - b0c), all SPMD-uniform.  fp8 noise on
the diag is ~3e-3/entry and averages out (~5e-6 on the loss).

Constants b0c/Z0 cancel exactly in the host reduction.
"""

import sys

for _p in ("/opt/trn_rl_repo",):
    if _p not in sys.path:
        sys.path.insert(0, _p)

import numpy as np
import ml_dtypes

import concourse.bass as bass
import concourse.bacc as bacc
import concourse.tile as tile
from concourse import mybir
from concourse import bass_utils

B = 4096
D = 2048
TEMP = 0.5
NCORES = 8
SHARD = B // NCORES  # 512 j-rows per core
NT = SHARD // 128  # 4 j partition-tiles
KT = D // 128  # 16 k partition-tiles
NCH = 8  # i chunks of 512

F32 = mybir.dt.float32
F16 = mybir.dt.float16
F8 = mybir.dt.float8e4
AF = mybir.ActivationFunctionType
OP = mybir.AluOpType
AX = mybir.AxisListType
PM = mybir.MatmulPerfMode

CW = 512.0  # W8 = CW*Q: typ ~0.25, max ~15; fp8e4 max is 240
SINV = 2.0 / CW  # S' * SINV = (ci.Q)/T
B0C = -2.0  # bias recenter: u = exp(G - 2lnZ - B0C) stays ~1 in fp16
Z0 = 3400.0  # zn2 = (Z/Z0)^2 ~ 1


def build_kernel_body(tc, zp_ap, gd_ap, zo_ap, cit_ap, cj_ap, eye_ap):
    nc = tc.nc
    from contextlib import ExitStack

    with ExitStack() as ctx:
        singles = ctx.enter_context(tc.tile_pool(name="singles", bufs=1))
        xin = ctx.enter_context(tc.tile_pool(name="xin", bufs=2))
        est = ctx.enter_context(tc.tile_pool(name="est", bufs=3))
        scr = ctx.enter_context(tc.tile_pool(name="scr", bufs=2))
        wst = ctx.enter_context(tc.tile_pool(name="wst", bufs=2))
        wtt = ctx.enter_context(tc.tile_pool(name="wtt", bufs=2))
        ups = ctx.enter_context(tc.tile_pool(name="ups", bufs=3))
        pps = ctx.enter_context(tc.tile_pool(name="pps", bufs=2))
        dps = ctx.enter_context(tc.tile_pool(name="dps", bufs=2))
        psS = ctx.enter_context(tc.tile_pool(name="psS", bufs=4, space="PSUM"))

        # resident tiles
        cit8 = singles.tile([128, KT, B], F8)  # 64KB/part
        wt8 = singles.tile([128, NT, KT, 128], F8)  # 8KB/part
        pacc = singles.tile([128, NCH, 512], F16)  # 8KB/part
        eye32 = singles.tile([128, 128], F32)
        Zc = singles.tile([128, NT], F32)
        Ac = singles.tile([128, NT], F32)
        rzc = singles.tile([128, NT], F32)
        scw = singles.tile([128, NT], F32)
        bb2 = singles.tile([128, NT], F32)
        zn2 = singles.tile([128, NT], F32)
        tmp1 = singles.tile([128, NT], F32)
        gd = singles.tile([128, NT], F32)

        # ── loads: three independent rings ────────────────────────────────
        # sync ring: the 8 ciT eighths (1MB each, 8KB/partition contiguous)
        for e in range(NCH):
            nc.sync.dma_start(
                out=cit8[:, :, 512 * e : 512 * (e + 1)],
                in_=cit_ap[128 * e : 128 * (e + 1), :],
            )
        # gpsimd ring: eye + the 4 cj tiles
        nc.gpsimd.dma_start(out=eye32, in_=eye_ap)
        cj_tiles = []
        for t in range(NT):
            cjt = xin.tile([128, D], F32, tag="cj", name=f"cjt{t}", bufs=4)
            nc.gpsimd.dma_start(out=cjt, in_=cj_ap[128 * t : 128 * (t + 1), :])
            cj_tiles.append(cjt)

        # ── prep for all 4 j-tiles (emitted before any stripes) ───────────
        for t in range(NT):
            cj_t = cj_tiles[t]
            E32 = est.tile([128, D], F32, tag="e32")
            nc.scalar.activation(
                out=E32, in_=cj_t, func=AF.Exp, accum_out=Zc[:, t : t + 1]
            )
            # A_j = sum E*cj (mul on gpsimd, reduce on DVE)
            j1 = scr.tile([128, D], F16, tag="j1")
            nc.gpsimd.tensor_mul(j1, E32, cj_t)
            nc.vector.tensor_reduce(
                out=Ac[:, t : t + 1], in_=j1, axis=AX.X, op=OP.add
            )
            # per-j scalars: rz = 1/Z, scw = CW/Z, bb2 = -2A/Z - B0C,
            # zn2 = (Z/Z0)^2
            nc.vector.reciprocal(out=rzc[:, t : t + 1], in_=Zc[:, t : t + 1])
            nc.vector.tensor_scalar_mul(
                scw[:, t : t + 1], rzc[:, t : t + 1], float(CW)
            )
            nc.vector.tensor_mul(bb2[:, t : t + 1], Ac[:, t : t + 1], rzc[:, t : t + 1])
            nc.vector.tensor_scalar(
                out=bb2[:, t : t + 1], in0=bb2[:, t : t + 1],
                scalar1=-2.0, scalar2=-float(B0C), op0=OP.mult, op1=OP.add,
            )
            nc.vector.tensor_scalar_mul(
                tmp1[:, t : t + 1], Zc[:, t : t + 1], float(1.0 / Z0)
            )
            nc.vector.tensor_mul(
                zn2[:, t : t + 1], tmp1[:, t : t + 1], tmp1[:, t : t + 1]
            )
            # W16 = E * (CW/Z_j) -> transpose (scalar ring) -> fp8
            w16 = wst.tile([128, D], F16, tag="w16")
            nc.vector.tensor_scalar_mul(w16, E32, scw[:, t : t + 1])
            wt16 = wtt.tile([128, KT, 128], F16, tag="wt16")
            nc.scalar.dma_start_transpose(out=wt16, in_=w16)
            nc.vector.tensor_copy(out=wt8[:, t], in_=wt16)

        # ── stripes: S'[j=128, i=512]; chunks 0-3 for all t, then 4-7 ─────
        for crange in (range(0, NCH // 2), range(NCH // 2, NCH)):
            for t in range(NT):
                for c2 in crange:
                    S_ps = psS.tile([128, 512], F32, tag="s")
                    for k2 in range(KT // 2):
                        nc.tensor.matmul(
                            S_ps,
                            wt8[:, t, 2 * k2 : 2 * k2 + 2, :],
                            cit8[:, 2 * k2 : 2 * k2 + 2, 512 * c2 : 512 * (c2 + 1)],
                            start=(k2 == 0),
                            stop=(k2 == KT // 2 - 1),
                            perf_mode=PM.DoubleRow,
                        )
                    if c2 == 0:
                        # diagonal: G_ii - b0c = diag(S')*SINV + bb2
                        junk = dps.tile([128, 128], F32, tag="junk")
                        nc.vector.tensor_mul(
                            junk, S_ps[:, 128 * t : 128 * (t + 1)], eye32
                        )
                        dd = dps.tile([128, 1], F32, tag="dd")
                        nc.vector.tensor_reduce(
                            out=dd, in_=junk, axis=AX.X, op=OP.add
                        )
                        nc.vector.tensor_scalar(
                            out=gd[:, t : t + 1], in0=dd,
                            scalar1=float(SINV), scalar2=bb2[:, t : t + 1],
                            op0=OP.mult, op1=OP.add,
                        )
                    u16 = ups.tile([128, 512], F16, tag="u16")
                    nc.scalar.activation(
                        out=u16, in_=S_ps, func=AF.Exp,
                        scale=float(SINV), bias=bb2[:, t : t + 1],
                    )
                    if t == 0:
                        nc.vector.tensor_scalar_mul(
                            pacc[:, c2], u16, zn2[:, t : t + 1]
                        )
                    else:
                        p16 = pps.tile([128, 512], F16, tag="p16")
                        nc.vector.tensor_scalar_mul(p16, u16, zn2[:, t : t + 1])
                        nc.vector.tensor_add(pacc[:, c2], pacc[:, c2], p16)
                    if t == NT - 1:
                        nc.sync.dma_start(
                            out=zp_ap[:, 512 * c2 : 512 * (c2 + 1)],
                            in_=pacc[:, c2],
                        )

        nc.sync.dma_start(out=gd_ap, in_=gd)
        nc.sync.dma_start(out=zo_ap, in_=Zc)


_NC_CACHE = {}


def build_nc():
    key = "nc"
    if key in _NC_CACHE:
        return _NC_CACHE[key]
    nc = bacc.Bacc("TRN2", target_bir_lowering=False, debug=False)
    cit = nc.dram_tensor("cit", [NCH * 128, KT * 512], F8, kind="ExternalInput").ap()
    cj = nc.dram_tensor("cj", [SHARD, D], F32, kind="ExternalInput").ap()
    eye = nc.dram_tensor("eye", [128, 128], F32, kind="ExternalInput").ap()
    zp = nc.dram_tensor("zp", [128, B], F16, kind="ExternalOutput").ap()
    gd = nc.dram_tensor("gd", [128, NT], F32, kind="ExternalOutput").ap()
    zo = nc.dram_tensor("zo", [128, NT], F32, kind="ExternalOutput").ap()
    with tile.TileContext(nc) as tc:
        build_kernel_body(tc, zp, gd, zo, cit, cj, eye)
    nc.compile()
    _NC_CACHE[key] = nc
    return nc


def make_in_maps(c_i, c_j):
    # eighth-major ciT: block e is rows [512e, 512e+512) of c_i as columns;
    # per core, roll blocks so local chunk 0 = the core's own j-range (the
    # diagonal block).
    cit8 = c_i.T.astype(ml_dtypes.float8_e4m3)  # [D, B] = [(kt p), (e i')]
    base = np.ascontiguousarray(
        cit8.reshape(KT, 128, NCH, 512).transpose(2, 1, 0, 3)
    )  # [e, p, kt, i']
    eye = np.eye(128, dtype=np.float32)
    in_maps = []
    for c in range(NCORES):
        rolled = np.ascontiguousarray(np.roll(base, -c, axis=0)).reshape(
            NCH * 128, KT * 512
        )
        in_maps.append(
            {
                "cit": rolled,
                "cj": np.ascontiguousarray(c_j[SHARD * c : SHARD * (c + 1)]),
                "eye": eye,
            }
        )
    return in_maps


def kernel(c_i, c_j, **kwargs):
    c_i = np.ascontiguousarray(np.asarray(c_i, dtype=np.float32))
    c_j = np.ascontiguousarray(np.asarray(c_j, dtype=np.float32))
    nc = build_nc()
    in_maps = make_in_maps(c_i, c_j)
    res = bass_utils.run_bass_kernel_spmd(
        nc, in_maps, core_ids=list(range(NCORES))
    )

    Zi = np.zeros(B, dtype=np.float64)
    gii_sum = np.float64(0.0)
    for c, r in enumerate(res.results):
        zl = r["zp"].astype(np.float64).sum(axis=0).reshape(NCH, 512)
        Zi += np.roll(zl, c, axis=0).reshape(-1)
        # G_ii = gd + 2 lnZ + b0c  (gd = S*SINV + bb2 lacks the 2 lnZ term)
        gii_sum += (
            r["gd"].astype(np.float64)
            + 2.0 * np.log(r["zo"].astype(np.float64))
            + B0C
        ).sum()
    lse_sum = np.log(Zi).sum() + B * (B0C + 2.0 * np.log(Z0))
    loss = (lse_sum - gii_sum) / B
    return np.float32(loss).reshape(())
